# revision 8
# baseline (speedup 1.0000x reference)
"""Mistral MoE layer (H=2048, F=8192, E=8, top-2) on 8 Trainium2 NeuronCores.

Strategy: tensor-parallel over the expert FFN intermediate dim (F-shard).
Each core owns a 1024-wide slice of F for ALL 8 experts and processes,
sequentially per expert, exactly the tokens routed to that expert:

  stage 1:  hT[f, c] = w1_slice.T x ;  uT[f, c] = w3_slice.T x
            yT[f, c] = silu(hT) * uT                     (bf16)
  stage 2:  partial_out[h, c] = w2_slice.T yT, scaled by the combined
            gate weight per token (folded into the PSUM-evacuation op)

The host sums the 8 per-core partial outputs and scatter-adds them into
the token-major output.  This gives perfect load balance (every core does
sum(L_e)/8 = 1024 token-pair-equivalents regardless of routing skew), no
capacity padding (moving dim = tokens, chunked to the real count), no
DRAM bounce accumulation (stage-2 contraction is only 8 f-tiles -> a
single PSUM accumulation group), and all-bf16 matmuls (rel err ~4e-3,
measured offline, vs the 2e-2 gate).
"""

import math

import numpy as np
import ml_dtypes

import concourse.bass as bass
import concourse.mybir as mybir
import concourse.tile as tile
from concourse import bacc
from concourse.bass_utils import run_bass_kernel_spmd

P = 128
H = 2048
F = 8192
E = 8
TOP_K = 2
N_CORES = 8
F_LOC = F // N_CORES          # 1024 — per-core F slice
N_FI = F_LOC // P             # 8 f-tiles per expert per core
N_HH = H // P                 # 16 contraction tiles over hidden dim
SEG_CAP = 1280                # max padded tokens per segment (SBUF budget)

_kernel_cache: dict = {}

# Test-harness knobs: when TRACE is true the SPMD run captures an NTFF
# profile and the BassKernelResults lands in LAST_RESULT.
TRACE = False
LAST_RESULT = None


def _chunks(L):
    """Split L tokens into matmul moving-dim chunks <= 512, multiple of 4."""
    n = max(1, math.ceil(L / 512))
    out = []
    rem = L
    for i in range(n):
        c = (rem // (n - i) + 3) // 4 * 4
        c = min(c, rem)
        out.append(c)
        rem -= c
    assert sum(out) == L and all(c <= 512 for c in out)
    return out


def build_kernel(seg_lens):
    """One core's program: per segment s (expert slot) of seg_lens[s] padded
    tokens, run the F-sliced SwiGLU FFN.  Returns finalized Bacc."""
    f32 = mybir.dt.float32
    bf16 = mybir.dt.bfloat16
    nseg = len(seg_lens)
    L_tot = sum(seg_lens)
    offs = np.concatenate([[0], np.cumsum(seg_lens)]).astype(int)

    nc = bacc.Bacc("TRN2", target_bir_lowering=False, debug=False)
    xt_d = nc.dram_tensor("xt", [H, L_tot], bf16, kind="ExternalInput")
    gw_d = nc.dram_tensor("gwb", [P, L_tot], f32, kind="ExternalInput")
    w1_d = nc.dram_tensor("w1s", [nseg * N_FI * P, H], bf16, kind="ExternalInput")
    w3_d = nc.dram_tensor("w3s", [nseg * N_FI * P, H], bf16, kind="ExternalInput")
    w2_d = nc.dram_tensor("w2s", [nseg * N_FI * P, H], bf16, kind="ExternalInput")
    out_d = nc.dram_tensor("out", [H, L_tot], f32, kind="ExternalOutput")

    xt_r = xt_d[:, :].rearrange("(ho hi) c -> hi ho c", hi=P)
    out_r = out_d[:, :].rearrange("(ht hp) c -> hp ht c", hp=P)

    with tile.TileContext(nc) as tc:
        with (
            tc.tile_pool(name="xpool", bufs=2) as xpool,
            tc.tile_pool(name="gpool", bufs=2) as gpool,
            tc.tile_pool(name="wpool", bufs=2) as wpool,
            tc.tile_pool(name="w2pool", bufs=1) as w2pool,
            tc.tile_pool(name="ypool", bufs=2) as ypool,
            tc.tile_pool(name="spool", bufs=2) as spool,
            tc.tile_pool(name="opool", bufs=2) as opool,
            tc.tile_pool(name="psum", bufs=1, space="PSUM") as psum,
        ):
            # Prefetched token/gate/weight tiles, one segment ahead (so the
            # DMAs for segment si+1 are enqueued before stage-2(si)'s output
            # DMAs fill the FIFO queues).
            xt_tiles: dict = {}
            gw_tiles: dict = {}
            w_tiles: dict = {}

            def fetch_seg(si, split_first=False):
                L = seg_lens[si]
                o = int(offs[si])
                xt_s = xpool.tile([P, N_HH, L], bf16, tag="xt", name=f"xt{si}")
                if split_first:
                    # per-chunk column DMAs so the PE can start on chunk 0
                    # without waiting for the whole segment to land
                    c0 = 0
                    for i, cw in enumerate(_chunks(L)):
                        eng = nc.sync if i % 2 == 0 else nc.scalar
                        eng.dma_start(
                            xt_s[:, :, c0 : c0 + cw],
                            xt_r[:, :, o + c0 : o + c0 + cw],
                        )
                        c0 += cw
                else:
                    nc.sync.dma_start(xt_s[:, 0:8, :], xt_r[:, 0:8, o : o + L])
                    nc.scalar.dma_start(xt_s[:, 8:16, :], xt_r[:, 8:16, o : o + L])
                gw_s = gpool.tile([P, L], f32, tag="gw", name=f"gw{si}")
                nc.scalar.dma_start(gw_s[:], gw_d[:, o : o + L])
                xt_tiles[si] = xt_s
                gw_tiles[si] = gw_s

            def fetch_wtile(si, fi):
                row = bass.ts(si * N_FI + fi, P)
                w1_t = wpool.tile([P, N_HH, P], bf16, tag="w1t", name="w1_t")
                nc.sync.dma_start(
                    w1_t[:], w1_d[row, :].rearrange("p (ho f) -> p ho f", f=P)
                )
                w3_t = wpool.tile([P, N_HH, P], bf16, tag="w3t", name="w3_t")
                nc.scalar.dma_start(
                    w3_t[:], w3_d[row, :].rearrange("p (ho f) -> p ho f", f=P)
                )
                w_tiles[(si, fi)] = (w1_t, w3_t)

            # first segment's first weight tiles go out before its tokens so
            # the PE can start as soon as the tokens land
            fetch_wtile(0, 0)
            fetch_seg(0, split_first=True)

            for si in range(nseg):
                L = seg_lens[si]
                o = int(offs[si])
                ch = _chunks(L)
                xt_s = xt_tiles.pop(si)
                gw_s = gw_tiles.pop(si)

                # ---- stage 1: yT[f, c] for the 8 f-tiles of this segment
                yt = ypool.tile([P, N_FI, L], bf16, tag="yt", name=f"yt{si}")
                for fi in range(N_FI):
                    if (si, fi) in w_tiles:
                        w1_t, w3_t = w_tiles.pop((si, fi))
                    else:
                        fetch_wtile(si, fi)
                        w1_t, w3_t = w_tiles.pop((si, fi))
                    c0 = 0
                    for cw in ch:
                        csl = slice(c0, c0 + cw)
                        ph = psum.tile([P, cw], f32, tag="ph", bufs=2, name="ph")
                        for hh in range(N_HH):
                            nc.tensor.matmul(
                                ph[:],
                                w1_t[:, hh, :],
                                xt_s[:, hh, csl],
                                start=(hh == 0),
                                stop=(hh == N_HH - 1),
                            )
                        pu = psum.tile([P, cw], f32, tag="pu", bufs=2, name="pu")
                        for hh in range(N_HH):
                            nc.tensor.matmul(
                                pu[:],
                                w3_t[:, hh, :],
                                xt_s[:, hh, csl],
                                start=(hh == 0),
                                stop=(hh == N_HH - 1),
                            )
                        sl = spool.tile([P, cw], f32, tag="sl", name="sl")
                        nc.scalar.activation(
                            sl[:], ph[:], mybir.ActivationFunctionType.Silu
                        )
                        nc.vector.tensor_tensor(
                            yt[:, fi, csl], sl[:], pu[:], mybir.AluOpType.mult
                        )
                        c0 += cw

                # prefetch next segment's tokens/gates and first two weight
                # tiles ahead of the out-DMA flood
                if si + 1 < nseg:
                    fetch_seg(si + 1)
                    fetch_wtile(si + 1, 0)
                    fetch_wtile(si + 1, 1)

                # ---- stage 2: partial down-projection, gate-scaled
                w2_t = w2pool.tile([P, N_FI, H], bf16, tag="w2t", name="w2_t")
                nc.scalar.dma_start(
                    w2_t[:],
                    w2_d[bass.ts(si, N_FI * P), :].rearrange("(f p) h -> p f h", p=P),
                )
                for ht in range(H // P):
                    ot = opool.tile([P, L], f32, tag="ot", bufs=3, name="ot")
                    c0 = 0
                    for cw in ch:
                        csl = slice(c0, c0 + cw)
                        po = psum.tile([P, cw], f32, tag="po", bufs=4, name="po")
                        for fi in range(N_FI):
                            nc.tensor.matmul(
                                po[:],
                                w2_t[:, fi, bass.ts(ht, P)],
                                yt[:, fi, csl],
                                start=(fi == 0),
                                stop=(fi == N_FI - 1),
                            )
                        nc.vector.tensor_tensor(
                            ot[:, csl], po[:], gw_s[:, csl], mybir.AluOpType.mult
                        )
                        c0 += cw
                    eng = nc.sync if ht % 2 == 0 else nc.scalar
                    eng.dma_start(out_r[:, ht, o : o + L], ot[:])
    nc.finalize()
    return nc


def _route(x, gate_w):
    """Host gate: top-2 + softmax.  Returns (xs, per-expert idx, weights)."""
    xs = x.reshape(-1, x.shape[-1])
    logits = xs.astype(np.float32) @ gate_w.astype(np.float32)  # [T, E]
    e1 = np.argmax(logits, axis=1)
    l1 = logits[np.arange(len(logits)), e1]
    masked = logits.copy()
    masked[np.arange(len(logits)), e1] = -np.inf
    e2 = np.argmax(masked, axis=1)
    l2 = masked[np.arange(len(logits)), e2]
    w_hi = 1.0 / (1.0 + np.exp(l2 - l1))
    w_lo = 1.0 - w_hi
    idxs, gws = [], []
    for e in range(E):
        sel1 = e1 == e
        sel2 = e2 == e
        idx = np.nonzero(sel1 | sel2)[0]
        w = np.where(sel1[idx], w_hi[idx], w_lo[idx]).astype(np.float32)
        idxs.append(idx)
        gws.append(w)
    return xs, idxs, gws


def _slice_weights(w1, w3, w2):
    """Pre-arrange weight slices for all cores.

    w1/w3 -> [E, 64, 128, 2048] bf16 where [e, fg, hi, (ho f)] =
             w[e, ho*128+hi, fg*128+f]   (fg = global f-tile index)
    w2    -> [E, 64, 128, 2048] bf16 where [e, fg, hi, h] =
             w2[e, fg*128+hi, h]
    """
    bf16 = ml_dtypes.bfloat16
    w1a = np.ascontiguousarray(
        w1.reshape(E, N_HH, P, F // P, P).transpose(0, 3, 2, 1, 4)
    ).reshape(E, F // P, P, H).astype(bf16)
    w3a = np.ascontiguousarray(
        w3.reshape(E, N_HH, P, F // P, P).transpose(0, 3, 2, 1, 4)
    ).reshape(E, F // P, P, H).astype(bf16)
    w2a = w2.reshape(E, F // P, P, H).astype(bf16)
    return w1a, w3a, w2a


def kernel(x, gate_w, w1, w3, w2):
    x = np.asarray(x)
    gate_w = np.asarray(gate_w)
    w1 = np.asarray(w1)
    w3 = np.asarray(w3)
    w2 = np.asarray(w2)
    bf16 = ml_dtypes.bfloat16

    xs, idxs, gws = _route(x, gate_w)
    T = xs.shape[0]

    # Build segments: (expert, token index array, padded length).  Experts
    # with more than SEG_CAP tokens are split into multiple segments.
    segs = []
    for e in range(E):
        idx = idxs[e]
        gw_e = gws[e]
        for s0 in range(0, max(len(idx), 1), SEG_CAP):
            part = idx[s0 : s0 + SEG_CAP]
            if len(part) == 0:
                continue
            Lp = (len(part) + 7) // 8 * 8
            segs.append((e, part, gw_e[s0 : s0 + len(part)], Lp))

    seg_lens = tuple(Lp for _, _, _, Lp in segs)
    if seg_lens not in _kernel_cache:
        _kernel_cache[seg_lens] = build_kernel(list(seg_lens))
    nc = _kernel_cache[seg_lens]

    L_tot = sum(seg_lens)
    offs = np.concatenate([[0], np.cumsum(seg_lens)]).astype(int)

    # Shared inputs: token matrix (transposed, bf16) and replicated gates.
    xt = np.zeros((H, L_tot), bf16)
    gwb_row = np.zeros(L_tot, np.float32)
    for si, (e, part, gw_e, Lp) in enumerate(segs):
        o = int(offs[si])
        xt[:, o : o + len(part)] = xs[part].T.astype(bf16)
        gwb_row[o : o + len(part)] = gw_e
    gwb = np.ascontiguousarray(np.broadcast_to(gwb_row, (P, L_tot)))

    w1a, w3a, w2a = _slice_weights(w1, w3, w2)

    in_maps = []
    for c in range(N_CORES):
        fsl = slice(c * N_FI, (c + 1) * N_FI)
        w1c = np.ascontiguousarray(w1a[:, fsl]).reshape(E * N_FI * P, H)
        w3c = np.ascontiguousarray(w3a[:, fsl]).reshape(E * N_FI * P, H)
        w2c = np.ascontiguousarray(w2a[:, fsl]).reshape(E * N_FI * P, H)
        # reorder rows to segment order (handles split segments)
        if len(segs) != E or any(si != segs[si][0] for si in range(len(segs))):
            rows1, rows3, rows2 = [], [], []
            for e, _, _, _ in segs:
                sl = slice(e * N_FI * P, (e + 1) * N_FI * P)
                rows1.append(w1c[sl])
                rows3.append(w3c[sl])
                rows2.append(w2c[sl])
            w1c = np.concatenate(rows1, axis=0)
            w3c = np.concatenate(rows3, axis=0)
            w2c = np.concatenate(rows2, axis=0)
        in_maps.append(
            {"xt": xt, "gwb": gwb, "w1s": w1c, "w3s": w3c, "w2s": w2c}
        )

    global LAST_RESULT
    if TRACE:
        try:
            res = run_bass_kernel_spmd(
                nc,
                in_maps,
                core_ids=list(range(N_CORES)),
                trace=True,
                trace_cores=list(range(N_CORES)),
            )
        except Exception as exc:
            import traceback

            print("TRACE FAILED:", exc)
            traceback.print_exc()
            res = run_bass_kernel_spmd(nc, in_maps, core_ids=list(range(N_CORES)))
    else:
        res = run_bass_kernel_spmd(nc, in_maps, core_ids=list(range(N_CORES)))
    LAST_RESULT = res

    out_sum = np.zeros((H, L_tot), np.float32)
    for c in range(N_CORES):
        out_sum += res.results[c]["out"]

    out_flat = np.zeros((T, H), np.float32)
    for si, (e, part, gw_e, Lp) in enumerate(segs):
        o = int(offs[si])
        out_flat[part] += out_sum[:, o : o + len(part)].T
    return out_flat.reshape(x.shape).astype(x.dtype)


# revision 12
# speedup vs baseline: 1.0685x; 1.0685x over previous
"""Mistral MoE layer (H=2048, F=8192, E=8, top-2) on 8 Trainium2 NeuronCores.

Strategy: tensor-parallel over the expert FFN intermediate dim (F-shard).
Each core owns a 1024-wide slice of F for ALL 8 experts and processes,
sequentially per expert, exactly the tokens routed to that expert:

  stage 1:  hT[f, c] = w1_slice.T x ;  uT[f, c] = w3_slice.T x
            yT[f, c] = silu(hT) * uT                     (bf16)
  stage 2:  partial_out[h, c] = w2_slice.T yT, scaled by the combined
            gate weight per token (folded into the PSUM-evacuation op)

The host sums the 8 per-core partial outputs and scatter-adds them into
the token-major output.  This gives perfect load balance (every core does
sum(L_e)/8 = 1024 token-pair-equivalents regardless of routing skew), no
capacity padding (moving dim = tokens, chunked to the real count), no
DRAM bounce accumulation (stage-2 contraction is only 8 f-tiles -> a
single PSUM accumulation group), and all-bf16 matmuls (rel err ~4e-3,
measured offline, vs the 2e-2 gate).
"""

import math

import numpy as np
import ml_dtypes

import concourse.bass as bass
import concourse.mybir as mybir
import concourse.tile as tile
from concourse import bacc
from concourse.bass_utils import run_bass_kernel_spmd

P = 128
H = 2048
F = 8192
E = 8
TOP_K = 2
N_CORES = 8
F_LOC = F // N_CORES          # 1024 — per-core F slice
N_FI = F_LOC // P             # 8 f-tiles per expert per core
N_HH = H // P                 # 16 contraction tiles over hidden dim
SEG_CAP = 1280                # max padded tokens per segment (SBUF budget)

_kernel_cache: dict = {}

# Test-harness knobs: when TRACE is true the SPMD run captures an NTFF
# profile and the BassKernelResults lands in LAST_RESULT.
TRACE = False
LAST_RESULT = None


def _chunks(L, small_first=False):
    """Split L tokens into matmul moving-dim chunks <= 512, multiple of 4.

    small_first carves a 128-column first chunk so the very first PSUM
    group only waits on a small slice of the token DMA (startup latency).
    """
    pre = []
    if small_first and L > 256:
        pre = [128]
        L -= 128
    n = max(1, math.ceil(L / 512))
    out = []
    rem = L
    for i in range(n):
        c = (rem // (n - i) + 3) // 4 * 4
        c = min(c, rem)
        out.append(c)
        rem -= c
    out = pre + out
    assert sum(out) == L + sum(pre) and all(c <= 512 for c in out)
    return out


def build_kernel(seg_lens):
    """One core's program: per segment s (expert slot) of seg_lens[s] padded
    tokens, run the F-sliced SwiGLU FFN.  Returns finalized Bacc."""
    f32 = mybir.dt.float32
    bf16 = mybir.dt.bfloat16
    nseg = len(seg_lens)
    L_tot = sum(seg_lens)
    offs = np.concatenate([[0], np.cumsum(seg_lens)]).astype(int)

    nc = bacc.Bacc("TRN2", target_bir_lowering=False, debug=False)
    xt_d = nc.dram_tensor("xt", [H, L_tot], bf16, kind="ExternalInput")
    gw_d = nc.dram_tensor("gwb", [P, L_tot], f32, kind="ExternalInput")
    w1_d = nc.dram_tensor("w1s", [nseg * N_FI * P, H], bf16, kind="ExternalInput")
    w3_d = nc.dram_tensor("w3s", [nseg * N_FI * P, H], bf16, kind="ExternalInput")
    w2_d = nc.dram_tensor("w2s", [nseg * N_FI * P, H], bf16, kind="ExternalInput")
    out_d = nc.dram_tensor("out", [H, L_tot], f32, kind="ExternalOutput")

    xt_r = xt_d[:, :].rearrange("(ho hi) c -> hi ho c", hi=P)
    out_r = out_d[:, :].rearrange("(ht hp) c -> hp ht c", hp=P)

    with tile.TileContext(nc) as tc:
        with (
            tc.tile_pool(name="xpool", bufs=2) as xpool,
            tc.tile_pool(name="gpool", bufs=2) as gpool,
            tc.tile_pool(name="wpool", bufs=2) as wpool,
            tc.tile_pool(name="w2pool", bufs=1) as w2pool,
            tc.tile_pool(name="ypool", bufs=2) as ypool,
            tc.tile_pool(name="spool", bufs=2) as spool,
            tc.tile_pool(name="opool", bufs=2) as opool,
            tc.tile_pool(name="psum", bufs=1, space="PSUM") as psum,
        ):
            # Prefetched token/gate/weight tiles, one segment ahead (so the
            # DMAs for segment si+1 are enqueued before stage-2(si)'s output
            # DMAs fill the FIFO queues).
            xt_tiles: dict = {}
            gw_tiles: dict = {}
            w_tiles: dict = {}

            def fetch_seg(si, split_first=False):
                L = seg_lens[si]
                o = int(offs[si])
                xt_s = xpool.tile([P, N_HH, L], bf16, tag="xt", name=f"xt{si}")
                if split_first:
                    # per-chunk column DMAs so the PE can start on chunk 0
                    # without waiting for the whole segment to land
                    c0 = 0
                    for i, cw in enumerate(_chunks(L, small_first=True)):
                        eng = nc.sync if i % 2 == 0 else nc.scalar
                        eng.dma_start(
                            xt_s[:, :, c0 : c0 + cw],
                            xt_r[:, :, o + c0 : o + c0 + cw],
                        )
                        c0 += cw
                else:
                    nc.sync.dma_start(xt_s[:, 0:8, :], xt_r[:, 0:8, o : o + L])
                    nc.scalar.dma_start(xt_s[:, 8:16, :], xt_r[:, 8:16, o : o + L])
                gw_s = gpool.tile([P, L], f32, tag="gw", name=f"gw{si}")
                nc.scalar.dma_start(gw_s[:], gw_d[:, o : o + L])
                xt_tiles[si] = xt_s
                gw_tiles[si] = gw_s

            def fetch_wtile(si, fi):
                row = bass.ts(si * N_FI + fi, P)
                w1_t = wpool.tile([P, N_HH, P], bf16, tag="w1t", name="w1_t")
                nc.sync.dma_start(
                    w1_t[:], w1_d[row, :].rearrange("p (ho f) -> p ho f", f=P)
                )
                w3_t = wpool.tile([P, N_HH, P], bf16, tag="w3t", name="w3_t")
                nc.scalar.dma_start(
                    w3_t[:], w3_d[row, :].rearrange("p (ho f) -> p ho f", f=P)
                )
                w_tiles[(si, fi)] = (w1_t, w3_t)

            # first segment's first weight tiles go out before its tokens so
            # the PE can start as soon as the tokens land
            fetch_wtile(0, 0)
            fetch_seg(0, split_first=True)

            for si in range(nseg):
                L = seg_lens[si]
                o = int(offs[si])
                ch = _chunks(L, small_first=(si == 0))
                xt_s = xt_tiles.pop(si)
                gw_s = gw_tiles.pop(si)

                # ---- stage 1: yT[f, c] for the 8 f-tiles of this segment
                yt = ypool.tile([P, N_FI, L], bf16, tag="yt", name=f"yt{si}")
                for fi in range(N_FI):
                    if (si, fi) in w_tiles:
                        w1_t, w3_t = w_tiles.pop((si, fi))
                    else:
                        fetch_wtile(si, fi)
                        w1_t, w3_t = w_tiles.pop((si, fi))
                    c0 = 0
                    for cw in ch:
                        csl = slice(c0, c0 + cw)
                        ph = psum.tile([P, cw], f32, tag="ph", bufs=2, name="ph")
                        for hh in range(N_HH):
                            nc.tensor.matmul(
                                ph[:],
                                w1_t[:, hh, :],
                                xt_s[:, hh, csl],
                                start=(hh == 0),
                                stop=(hh == N_HH - 1),
                            )
                        pu = psum.tile([P, cw], f32, tag="pu", bufs=2, name="pu")
                        for hh in range(N_HH):
                            nc.tensor.matmul(
                                pu[:],
                                w3_t[:, hh, :],
                                xt_s[:, hh, csl],
                                start=(hh == 0),
                                stop=(hh == N_HH - 1),
                            )
                        sl = spool.tile([P, cw], f32, tag="sl", name="sl")
                        nc.scalar.activation(
                            sl[:], ph[:], mybir.ActivationFunctionType.Silu
                        )
                        nc.vector.tensor_tensor(
                            yt[:, fi, csl], sl[:], pu[:], mybir.AluOpType.mult
                        )
                        c0 += cw

                # prefetch next segment's tokens/gates and first two weight
                # tiles ahead of the out-DMA flood
                if si + 1 < nseg:
                    fetch_seg(si + 1)
                    fetch_wtile(si + 1, 0)
                    fetch_wtile(si + 1, 1)

                # ---- stage 2: partial down-projection, gate-scaled
                w2_t = w2pool.tile([P, N_FI, H], bf16, tag="w2t", name="w2_t")
                nc.scalar.dma_start(
                    w2_t[:],
                    w2_d[bass.ts(si, N_FI * P), :].rearrange("(f p) h -> p f h", p=P),
                )
                for ht in range(H // P):
                    ot = opool.tile([P, L], f32, tag="ot", bufs=4, name="ot")
                    c0 = 0
                    for cw in ch:
                        csl = slice(c0, c0 + cw)
                        po = psum.tile([P, cw], f32, tag="po", bufs=4, name="po")
                        for fi in range(N_FI):
                            nc.tensor.matmul(
                                po[:],
                                w2_t[:, fi, bass.ts(ht, P)],
                                yt[:, fi, csl],
                                start=(fi == 0),
                                stop=(fi == N_FI - 1),
                            )
                        nc.vector.tensor_tensor(
                            ot[:, csl], po[:], gw_s[:, csl], mybir.AluOpType.mult
                        )
                        c0 += cw
                    eng = nc.sync if ht % 2 == 0 else nc.scalar
                    eng.dma_start(out_r[:, ht, o : o + L], ot[:])
    nc.finalize()
    return nc


def _route(x, gate_w):
    """Host gate: top-2 + softmax.  Returns (xs, per-expert idx, weights)."""
    xs = x.reshape(-1, x.shape[-1])
    logits = xs.astype(np.float32) @ gate_w.astype(np.float32)  # [T, E]
    e1 = np.argmax(logits, axis=1)
    l1 = logits[np.arange(len(logits)), e1]
    masked = logits.copy()
    masked[np.arange(len(logits)), e1] = -np.inf
    e2 = np.argmax(masked, axis=1)
    l2 = masked[np.arange(len(logits)), e2]
    w_hi = 1.0 / (1.0 + np.exp(l2 - l1))
    w_lo = 1.0 - w_hi
    idxs, gws = [], []
    for e in range(E):
        sel1 = e1 == e
        sel2 = e2 == e
        idx = np.nonzero(sel1 | sel2)[0]
        w = np.where(sel1[idx], w_hi[idx], w_lo[idx]).astype(np.float32)
        idxs.append(idx)
        gws.append(w)
    return xs, idxs, gws


def _slice_weights(w1, w3, w2):
    """Pre-arrange weight slices for all cores.

    w1/w3 -> [E, 64, 128, 2048] bf16 where [e, fg, hi, (ho f)] =
             w[e, ho*128+hi, fg*128+f]   (fg = global f-tile index)
    w2    -> [E, 64, 128, 2048] bf16 where [e, fg, hi, h] =
             w2[e, fg*128+hi, h]
    """
    bf16 = ml_dtypes.bfloat16
    w1a = np.ascontiguousarray(
        w1.reshape(E, N_HH, P, F // P, P).transpose(0, 3, 2, 1, 4)
    ).reshape(E, F // P, P, H).astype(bf16)
    w3a = np.ascontiguousarray(
        w3.reshape(E, N_HH, P, F // P, P).transpose(0, 3, 2, 1, 4)
    ).reshape(E, F // P, P, H).astype(bf16)
    w2a = w2.reshape(E, F // P, P, H).astype(bf16)
    return w1a, w3a, w2a


def kernel(x, gate_w, w1, w3, w2):
    x = np.asarray(x)
    gate_w = np.asarray(gate_w)
    w1 = np.asarray(w1)
    w3 = np.asarray(w3)
    w2 = np.asarray(w2)
    bf16 = ml_dtypes.bfloat16

    xs, idxs, gws = _route(x, gate_w)
    T = xs.shape[0]

    # Build segments: (expert, token index array, padded length).  Experts
    # with more than SEG_CAP tokens are split into multiple segments.
    segs = []
    for e in range(E):
        idx = idxs[e]
        gw_e = gws[e]
        for s0 in range(0, max(len(idx), 1), SEG_CAP):
            part = idx[s0 : s0 + SEG_CAP]
            if len(part) == 0:
                continue
            Lp = (len(part) + 7) // 8 * 8
            segs.append((e, part, gw_e[s0 : s0 + len(part)], Lp))

    seg_lens = tuple(Lp for _, _, _, Lp in segs)
    if seg_lens not in _kernel_cache:
        _kernel_cache[seg_lens] = build_kernel(list(seg_lens))
    nc = _kernel_cache[seg_lens]

    L_tot = sum(seg_lens)
    offs = np.concatenate([[0], np.cumsum(seg_lens)]).astype(int)

    # Shared inputs: token matrix (transposed, bf16) and replicated gates.
    xt = np.zeros((H, L_tot), bf16)
    gwb_row = np.zeros(L_tot, np.float32)
    for si, (e, part, gw_e, Lp) in enumerate(segs):
        o = int(offs[si])
        xt[:, o : o + len(part)] = xs[part].T.astype(bf16)
        gwb_row[o : o + len(part)] = gw_e
    gwb = np.ascontiguousarray(np.broadcast_to(gwb_row, (P, L_tot)))

    w1a, w3a, w2a = _slice_weights(w1, w3, w2)

    in_maps = []
    for c in range(N_CORES):
        fsl = slice(c * N_FI, (c + 1) * N_FI)
        w1c = np.ascontiguousarray(w1a[:, fsl]).reshape(E * N_FI * P, H)
        w3c = np.ascontiguousarray(w3a[:, fsl]).reshape(E * N_FI * P, H)
        w2c = np.ascontiguousarray(w2a[:, fsl]).reshape(E * N_FI * P, H)
        # reorder rows to segment order (handles split segments)
        if len(segs) != E or any(si != segs[si][0] for si in range(len(segs))):
            rows1, rows3, rows2 = [], [], []
            for e, _, _, _ in segs:
                sl = slice(e * N_FI * P, (e + 1) * N_FI * P)
                rows1.append(w1c[sl])
                rows3.append(w3c[sl])
                rows2.append(w2c[sl])
            w1c = np.concatenate(rows1, axis=0)
            w3c = np.concatenate(rows3, axis=0)
            w2c = np.concatenate(rows2, axis=0)
        in_maps.append(
            {"xt": xt, "gwb": gwb, "w1s": w1c, "w3s": w3c, "w2s": w2c}
        )

    global LAST_RESULT
    if TRACE:
        try:
            res = run_bass_kernel_spmd(
                nc,
                in_maps,
                core_ids=list(range(N_CORES)),
                trace=True,
                trace_cores=list(range(N_CORES)),
            )
        except Exception as exc:
            import traceback

            print("TRACE FAILED:", exc)
            traceback.print_exc()
            res = run_bass_kernel_spmd(nc, in_maps, core_ids=list(range(N_CORES)))
    else:
        res = run_bass_kernel_spmd(nc, in_maps, core_ids=list(range(N_CORES)))
    LAST_RESULT = res

    out_sum = np.zeros((H, L_tot), np.float32)
    for c in range(N_CORES):
        out_sum += res.results[c]["out"]

    out_flat = np.zeros((T, H), np.float32)
    for si, (e, part, gw_e, Lp) in enumerate(segs):
        o = int(offs[si])
        out_flat[part] += out_sum[:, o : o + len(part)].T
    return out_flat.reshape(x.shape).astype(x.dtype)


# revision 14
# speedup vs baseline: 1.0826x; 1.0131x over previous
"""Mistral MoE layer (H=2048, F=8192, E=8, top-2) on 8 Trainium2 NeuronCores.

Strategy: tensor-parallel over the expert FFN intermediate dim (F-shard).
Each core owns a 1024-wide slice of F for ALL 8 experts and processes,
sequentially per expert, exactly the tokens routed to that expert:

  stage 1:  hT[f, c] = w1_slice.T x ;  uT[f, c] = w3_slice.T x
            yT[f, c] = silu(hT) * uT                     (bf16)
  stage 2:  partial_out[h, c] = w2_slice.T yT, scaled by the combined
            gate weight per token (folded into the PSUM-evacuation op)

The host sums the 8 per-core partial outputs and scatter-adds them into
the token-major output.  This gives perfect load balance (every core does
sum(L_e)/8 = 1024 token-pair-equivalents regardless of routing skew), no
capacity padding (moving dim = tokens, chunked to the real count), no
DRAM bounce accumulation (stage-2 contraction is only 8 f-tiles -> a
single PSUM accumulation group), and all-bf16 matmuls (rel err ~4e-3,
measured offline, vs the 2e-2 gate).
"""

import math

import numpy as np
import ml_dtypes

import concourse.bass as bass
import concourse.mybir as mybir
import concourse.tile as tile
from concourse import bacc
from concourse.bass_utils import run_bass_kernel_spmd

P = 128
H = 2048
F = 8192
E = 8
TOP_K = 2
N_CORES = 8
F_LOC = F // N_CORES          # 1024 — per-core F slice
N_FI = F_LOC // P             # 8 f-tiles per expert per core
N_HH = H // P                 # 16 contraction tiles over hidden dim
SEG_CAP = 1280                # max padded tokens per segment (SBUF budget)

_kernel_cache: dict = {}

# Test-harness knobs: when TRACE is true the SPMD run captures an NTFF
# profile and the BassKernelResults lands in LAST_RESULT.
TRACE = False
LAST_RESULT = None


def _chunks(L, small_first=False):
    """Split L tokens into matmul moving-dim chunks <= 512, multiple of 4.

    small_first carves a 128-column first chunk so the very first PSUM
    group only waits on a small slice of the token DMA (startup latency).
    """
    pre = []
    if small_first and L > 256:
        pre = [128]
        L -= 128
    n = max(1, math.ceil(L / 512))
    out = []
    rem = L
    for i in range(n):
        c = (rem // (n - i) + 3) // 4 * 4
        c = min(c, rem)
        out.append(c)
        rem -= c
    out = pre + out
    assert sum(out) == L + sum(pre) and all(c <= 512 for c in out)
    return out


def build_kernel(seg_lens):
    """One core's program: per segment s (expert slot) of seg_lens[s] padded
    tokens, run the F-sliced SwiGLU FFN.  Returns finalized Bacc."""
    f32 = mybir.dt.float32
    bf16 = mybir.dt.bfloat16
    nseg = len(seg_lens)
    L_tot = sum(seg_lens)
    offs = np.concatenate([[0], np.cumsum(seg_lens)]).astype(int)

    nc = bacc.Bacc("TRN2", target_bir_lowering=False, debug=False)
    xt_d = nc.dram_tensor("xt", [H, L_tot], bf16, kind="ExternalInput")
    gw_d = nc.dram_tensor("gwb", [P, L_tot], f32, kind="ExternalInput")
    w1_d = nc.dram_tensor("w1s", [nseg * N_FI * P, H], bf16, kind="ExternalInput")
    w3_d = nc.dram_tensor("w3s", [nseg * N_FI * P, H], bf16, kind="ExternalInput")
    w2_d = nc.dram_tensor("w2s", [nseg * N_FI * P, H], bf16, kind="ExternalInput")
    out_d = nc.dram_tensor("out", [H, L_tot], f32, kind="ExternalOutput")

    xt_r = xt_d[:, :].rearrange("(ho hi) c -> hi ho c", hi=P)
    out_r = out_d[:, :].rearrange("(ht hp) c -> hp ht c", hp=P)

    with tile.TileContext(nc) as tc:
        with (
            tc.tile_pool(name="xpool", bufs=2) as xpool,
            tc.tile_pool(name="gpool", bufs=2) as gpool,
            tc.tile_pool(name="wpool", bufs=2) as wpool,
            tc.tile_pool(name="w2pool", bufs=1) as w2pool,
            tc.tile_pool(name="ypool", bufs=2) as ypool,
            tc.tile_pool(name="spool", bufs=2) as spool,
            tc.tile_pool(name="opool", bufs=2) as opool,
            tc.tile_pool(name="psum", bufs=1, space="PSUM") as psum,
        ):
            # Prefetched token/gate/weight tiles, one segment ahead (so the
            # DMAs for segment si+1 are enqueued before stage-2(si)'s output
            # DMAs fill the FIFO queues).
            xt_tiles: dict = {}
            gw_tiles: dict = {}
            w_tiles: dict = {}

            def fetch_seg(si, split_first=False):
                L = seg_lens[si]
                o = int(offs[si])
                xt_s = xpool.tile([P, N_HH, L], bf16, tag="xt", name=f"xt{si}")
                if split_first:
                    # per-chunk column DMAs, each split across both queues,
                    # so the PE can start on chunk 0 without waiting for the
                    # whole segment to land
                    c0 = 0
                    for cw in _chunks(L, small_first=True):
                        nc.sync.dma_start(
                            xt_s[:, 0:8, c0 : c0 + cw],
                            xt_r[:, 0:8, o + c0 : o + c0 + cw],
                        )
                        nc.scalar.dma_start(
                            xt_s[:, 8:16, c0 : c0 + cw],
                            xt_r[:, 8:16, o + c0 : o + c0 + cw],
                        )
                        c0 += cw
                else:
                    nc.sync.dma_start(xt_s[:, 0:8, :], xt_r[:, 0:8, o : o + L])
                    nc.scalar.dma_start(xt_s[:, 8:16, :], xt_r[:, 8:16, o : o + L])
                gw_s = gpool.tile([P, L], f32, tag="gw", name=f"gw{si}")
                nc.scalar.dma_start(gw_s[:], gw_d[:, o : o + L])
                xt_tiles[si] = xt_s
                gw_tiles[si] = gw_s

            def fetch_wtile(si, fi):
                row = bass.ts(si * N_FI + fi, P)
                w1_t = wpool.tile([P, N_HH, P], bf16, tag="w1t", bufs=3, name="w1_t")
                nc.sync.dma_start(
                    w1_t[:], w1_d[row, :].rearrange("p (ho f) -> p ho f", f=P)
                )
                w3_t = wpool.tile([P, N_HH, P], bf16, tag="w3t", bufs=3, name="w3_t")
                nc.scalar.dma_start(
                    w3_t[:], w3_d[row, :].rearrange("p (ho f) -> p ho f", f=P)
                )
                w_tiles[(si, fi)] = (w1_t, w3_t)

            # first segment's first weight tiles go out before its tokens so
            # the PE can start as soon as the tokens land
            fetch_wtile(0, 0)
            fetch_seg(0, split_first=True)

            for si in range(nseg):
                L = seg_lens[si]
                o = int(offs[si])
                ch = _chunks(L, small_first=(si == 0))
                xt_s = xt_tiles.pop(si)
                gw_s = gw_tiles.pop(si)

                # ---- stage 1: yT[f, c] for the 8 f-tiles of this segment
                yt = ypool.tile([P, N_FI, L], bf16, tag="yt", name=f"yt{si}")
                for fi in range(N_FI):
                    if (si, fi) in w_tiles:
                        w1_t, w3_t = w_tiles.pop((si, fi))
                    else:
                        fetch_wtile(si, fi)
                        w1_t, w3_t = w_tiles.pop((si, fi))
                    c0 = 0
                    for cw in ch:
                        csl = slice(c0, c0 + cw)
                        ph = psum.tile([P, cw], f32, tag="ph", bufs=2, name="ph")
                        for hh in range(N_HH):
                            nc.tensor.matmul(
                                ph[:],
                                w1_t[:, hh, :],
                                xt_s[:, hh, csl],
                                start=(hh == 0),
                                stop=(hh == N_HH - 1),
                            )
                        pu = psum.tile([P, cw], f32, tag="pu", bufs=2, name="pu")
                        for hh in range(N_HH):
                            nc.tensor.matmul(
                                pu[:],
                                w3_t[:, hh, :],
                                xt_s[:, hh, csl],
                                start=(hh == 0),
                                stop=(hh == N_HH - 1),
                            )
                        sl = spool.tile([P, cw], f32, tag="sl", name="sl")
                        nc.scalar.activation(
                            sl[:], ph[:], mybir.ActivationFunctionType.Silu
                        )
                        nc.vector.tensor_tensor(
                            yt[:, fi, csl], sl[:], pu[:], mybir.AluOpType.mult
                        )
                        c0 += cw

                # prefetch next segment's tokens/gates and first two weight
                # tiles ahead of the out-DMA flood
                if si + 1 < nseg:
                    fetch_seg(si + 1)
                    fetch_wtile(si + 1, 0)
                    fetch_wtile(si + 1, 1)

                # ---- stage 2: partial down-projection, gate-scaled
                w2_t = w2pool.tile([P, N_FI, H], bf16, tag="w2t", name="w2_t")
                nc.scalar.dma_start(
                    w2_t[:],
                    w2_d[bass.ts(si, N_FI * P), :].rearrange("(f p) h -> p f h", p=P),
                )
                for ht in range(H // P):
                    ot = opool.tile([P, L], f32, tag="ot", bufs=4, name="ot")
                    c0 = 0
                    for cw in ch:
                        csl = slice(c0, c0 + cw)
                        po = psum.tile([P, cw], f32, tag="po", bufs=4, name="po")
                        for fi in range(N_FI):
                            nc.tensor.matmul(
                                po[:],
                                w2_t[:, fi, bass.ts(ht, P)],
                                yt[:, fi, csl],
                                start=(fi == 0),
                                stop=(fi == N_FI - 1),
                            )
                        nc.vector.tensor_tensor(
                            ot[:, csl], po[:], gw_s[:, csl], mybir.AluOpType.mult
                        )
                        c0 += cw
                    eng = nc.sync if ht % 2 == 0 else nc.scalar
                    eng.dma_start(out_r[:, ht, o : o + L], ot[:])
    nc.finalize()
    return nc


def _route(x, gate_w):
    """Host gate: top-2 + softmax.  Returns (xs, per-expert idx, weights)."""
    xs = x.reshape(-1, x.shape[-1])
    logits = xs.astype(np.float32) @ gate_w.astype(np.float32)  # [T, E]
    e1 = np.argmax(logits, axis=1)
    l1 = logits[np.arange(len(logits)), e1]
    masked = logits.copy()
    masked[np.arange(len(logits)), e1] = -np.inf
    e2 = np.argmax(masked, axis=1)
    l2 = masked[np.arange(len(logits)), e2]
    w_hi = 1.0 / (1.0 + np.exp(l2 - l1))
    w_lo = 1.0 - w_hi
    idxs, gws = [], []
    for e in range(E):
        sel1 = e1 == e
        sel2 = e2 == e
        idx = np.nonzero(sel1 | sel2)[0]
        w = np.where(sel1[idx], w_hi[idx], w_lo[idx]).astype(np.float32)
        idxs.append(idx)
        gws.append(w)
    return xs, idxs, gws


def _slice_weights(w1, w3, w2):
    """Pre-arrange weight slices for all cores.

    w1/w3 -> [E, 64, 128, 2048] bf16 where [e, fg, hi, (ho f)] =
             w[e, ho*128+hi, fg*128+f]   (fg = global f-tile index)
    w2    -> [E, 64, 128, 2048] bf16 where [e, fg, hi, h] =
             w2[e, fg*128+hi, h]
    """
    bf16 = ml_dtypes.bfloat16
    w1a = np.ascontiguousarray(
        w1.reshape(E, N_HH, P, F // P, P).transpose(0, 3, 2, 1, 4)
    ).reshape(E, F // P, P, H).astype(bf16)
    w3a = np.ascontiguousarray(
        w3.reshape(E, N_HH, P, F // P, P).transpose(0, 3, 2, 1, 4)
    ).reshape(E, F // P, P, H).astype(bf16)
    w2a = w2.reshape(E, F // P, P, H).astype(bf16)
    return w1a, w3a, w2a


def kernel(x, gate_w, w1, w3, w2):
    x = np.asarray(x)
    gate_w = np.asarray(gate_w)
    w1 = np.asarray(w1)
    w3 = np.asarray(w3)
    w2 = np.asarray(w2)
    bf16 = ml_dtypes.bfloat16

    xs, idxs, gws = _route(x, gate_w)
    T = xs.shape[0]

    # Build segments: (expert, token index array, padded length).  Experts
    # with more than SEG_CAP tokens are split into multiple segments.
    segs = []
    for e in range(E):
        idx = idxs[e]
        gw_e = gws[e]
        for s0 in range(0, max(len(idx), 1), SEG_CAP):
            part = idx[s0 : s0 + SEG_CAP]
            if len(part) == 0:
                continue
            Lp = (len(part) + 7) // 8 * 8
            segs.append((e, part, gw_e[s0 : s0 + len(part)], Lp))

    seg_lens = tuple(Lp for _, _, _, Lp in segs)
    if seg_lens not in _kernel_cache:
        _kernel_cache[seg_lens] = build_kernel(list(seg_lens))
    nc = _kernel_cache[seg_lens]

    L_tot = sum(seg_lens)
    offs = np.concatenate([[0], np.cumsum(seg_lens)]).astype(int)

    # Shared inputs: token matrix (transposed, bf16) and replicated gates.
    xt = np.zeros((H, L_tot), bf16)
    gwb_row = np.zeros(L_tot, np.float32)
    for si, (e, part, gw_e, Lp) in enumerate(segs):
        o = int(offs[si])
        xt[:, o : o + len(part)] = xs[part].T.astype(bf16)
        gwb_row[o : o + len(part)] = gw_e
    gwb = np.ascontiguousarray(np.broadcast_to(gwb_row, (P, L_tot)))

    w1a, w3a, w2a = _slice_weights(w1, w3, w2)

    in_maps = []
    for c in range(N_CORES):
        fsl = slice(c * N_FI, (c + 1) * N_FI)
        w1c = np.ascontiguousarray(w1a[:, fsl]).reshape(E * N_FI * P, H)
        w3c = np.ascontiguousarray(w3a[:, fsl]).reshape(E * N_FI * P, H)
        w2c = np.ascontiguousarray(w2a[:, fsl]).reshape(E * N_FI * P, H)
        # reorder rows to segment order (handles split segments)
        if len(segs) != E or any(si != segs[si][0] for si in range(len(segs))):
            rows1, rows3, rows2 = [], [], []
            for e, _, _, _ in segs:
                sl = slice(e * N_FI * P, (e + 1) * N_FI * P)
                rows1.append(w1c[sl])
                rows3.append(w3c[sl])
                rows2.append(w2c[sl])
            w1c = np.concatenate(rows1, axis=0)
            w3c = np.concatenate(rows3, axis=0)
            w2c = np.concatenate(rows2, axis=0)
        in_maps.append(
            {"xt": xt, "gwb": gwb, "w1s": w1c, "w3s": w3c, "w2s": w2c}
        )

    global LAST_RESULT
    if TRACE:
        try:
            res = run_bass_kernel_spmd(
                nc,
                in_maps,
                core_ids=list(range(N_CORES)),
                trace=True,
                trace_cores=list(range(N_CORES)),
            )
        except Exception as exc:
            import traceback

            print("TRACE FAILED:", exc)
            traceback.print_exc()
            res = run_bass_kernel_spmd(nc, in_maps, core_ids=list(range(N_CORES)))
    else:
        res = run_bass_kernel_spmd(nc, in_maps, core_ids=list(range(N_CORES)))
    LAST_RESULT = res

    out_sum = np.zeros((H, L_tot), np.float32)
    for c in range(N_CORES):
        out_sum += res.results[c]["out"]

    out_flat = np.zeros((T, H), np.float32)
    for si, (e, part, gw_e, Lp) in enumerate(segs):
        o = int(offs[si])
        out_flat[part] += out_sum[:, o : o + len(part)].T
    return out_flat.reshape(x.shape).astype(x.dtype)
